# revision 14
# baseline (speedup 1.0000x reference)
"""Bass/Trainium2 kernel v6 for nn_GCL_49959059587771 (GCL JSD loss).

Math (see reference): per node i and pair p with sorted batch index b_i:
    s_i = <z_i, gn_self[b_i]> / ||z_i||
    c_i = <z_i, gn_cross[b_i]> / ||z_i||
    d_i = softplus(-c_i) - softplus(-s_i)
    answer = sqrt(sum d1_i^2) + sqrt(sum d2_i^2)

Strategy:
  - nodes sharded 8 ways (6250/core, padded to 7 chunks of 1024)
  - ALL per-chunk inputs (transposed fp8 z, one-hot mask, g-window weights,
    reduce weights) packed into ONE byte tensor, staged DMAs (small first)
  - per chunk: DoubleRow fp8 matmuls (K=256, shared weight loads across the
    two 512-node halves) compute all window sims into a 2-bank PSUM tile;
    one DVE op applies the one-hot mask fused with the PSUM evac; one-hot
    reduce matmuls accumulate s/c rows and (from fp8 squared z) norm rows
    into persistent PSUM banks laid out [pair*14 + chunk*2 + half, 512]
  - short Ln/Exp epilogue; host sums 8x28 partials, sqrt, add.
"""

import numpy as np
from contextlib import ExitStack

import concourse.bass as bass
import concourse.bacc as bacc
import concourse.tile as tile
import concourse.mybir as mybir
from concourse.bass_utils import run_bass_kernel_spmd

N, G, D = 50000, 512, 256
NCORES = 8
RPC = N // NCORES            # 6250 nodes per core
CH = 1024                    # nodes per chunk
NCH = 7                      # chunks per core
NODES = NCH * CH             # 7168 padded
W = 32                       # per-chunk batch-value window (per pair)
HCH = 512                    # half-chunk (one PSUM bank / acc column span)

# packed per-chunk per-partition byte layout (weights first: the initial
# small DMA [0:2560] of chunk 0 unblocks its first P matmul early)
OFF_GW1 = 0                  # f8 [2, 128]
OFF_GW2 = 256
OFF_Z1 = 512                 # f8 [2, 1024]
OFF_Z2 = 2560
OFF_OH = 4608                # f8 [1024]
OFF_RWS = 5632               # bf16 [2(half), 28]
OFF_RWC = 5744
OFF_RN1 = 5856               # f8 [2(half), 2(k), 32]
OFF_RN2 = 5984
CHB = 6112                   # bytes per chunk per partition

AF = mybir.ActivationFunctionType
ALU = mybir.AluOpType
F32 = mybir.dt.float32
BF16 = mybir.dt.bfloat16
F8 = mybir.dt.float8e4
DR = mybir.MatmulPerfMode.DoubleRow

NR = 28                      # accumulator rows: pair*14 + chunk*2 + half


def build(debug=False):
    nc = bacc.Bacc("TRN2", target_bir_lowering=False, debug=debug)

    pkt = nc.dram_tensor("pkt", [128, NCH * CHB], F8, kind="ExternalInput")
    acc = nc.dram_tensor("acc", [NR, 2], F32, kind="ExternalOutput")

    with tile.TileContext(nc) as tc, ExitStack() as ctx:
        singles = ctx.enter_context(tc.tile_pool(name="singles", bufs=1))
        sqpool = ctx.enter_context(tc.tile_pool(name="sq", bufs=4))
        mkpool = ctx.enter_context(tc.tile_pool(name="mk", bufs=4))
        ppool = ctx.enter_context(tc.tile_pool(name="pp", bufs=2, space="PSUM"))
        apool = ctx.enter_context(tc.tile_pool(name="acc", bufs=1, space="PSUM"))

        acc_s = apool.tile([NR, HCH], F32)
        acc_c = apool.tile([NR, HCH], F32)
        acc_n = apool.tile([32, HCH], F32)

        eps_b = singles.tile([NR, 1], F32)
        nc.vector.memset(eps_b[:], 1e-12)
        one_b = singles.tile([NR, 1], F32)
        nc.vector.memset(one_b[:], 1.0)
        # preload the natural_log_exp ACT table set early (overlaps DMA)
        dummy = singles.tile([NR, 1], F32)
        nc.scalar.activation(out=dummy[:], in_=eps_b[:], func=AF.Ln, bias=1.0)

        # whole packed input resident in SBUF; staged DMAs, small first so
        # compute starts early, large later for DMA efficiency
        pk = singles.tile([128, NCH * CHB], F8)
        for b0, b1 in ((0, 1), (1, 2), (2, 4), (4, NCH)):
            nc.sync.dma_start(pk[:, b0 * CHB:b1 * CHB],
                              pkt[:, b0 * CHB:b1 * CHB])

        # software-pipelined issue: reduce-MMs for chunk c are issued one
        # chunk later so the PE queue never stalls on this chunk's DVE/ACT
        stage = {}
        for c in range(NCH + 1):
            if c < NCH:
                o = c * CHB
                z1c = pk[:, o + OFF_Z1:o + OFF_Z1 + 2048].rearrange(
                    "p (k n) -> p k n", k=2)
                z2c = pk[:, o + OFF_Z2:o + OFF_Z2 + 2048].rearrange(
                    "p (k n) -> p k n", k=2)
                ohc = pk[:, o + OFF_OH:o + OFF_OH + CH]
                gw1c = pk[:, o + OFF_GW1:o + OFF_GW1 + 256].rearrange(
                    "p (k n) -> p k n", k=2)
                gw2c = pk[:, o + OFF_GW2:o + OFF_GW2 + 256].rearrange(
                    "p (k n) -> p k n", k=2)
                rwsc = pk[:, o + OFF_RWS:o + OFF_RWS + 112].bitcast(
                    BF16).rearrange("p (h n) -> p h n", h=2)
                rwcc = pk[:, o + OFF_RWC:o + OFF_RWC + 112].bitcast(
                    BF16).rearrange("p (h n) -> p h n", h=2)
                rn1c = pk[:, o + OFF_RN1:o + OFF_RN1 + 128].rearrange(
                    "p (h k n) -> p h k n", h=2, k=2)
                rn2c = pk[:, o + OFF_RN2:o + OFF_RN2 + 128].rearrange(
                    "p (h k n) -> p h k n", h=2, k=2)

                # ---- P matmuls (PE, DoubleRow fp8, K=256); gw LDW shared
                # by the two half-chunk MMs ----
                pc = ppool.tile([128, CH], F32, tag="p")
                for h in range(2):
                    nc.tensor.matmul(pc[:, h * HCH:(h + 1) * HCH], gw1c,
                                     z1c[:, :, h * HCH:(h + 1) * HCH],
                                     start=True, stop=False, perf_mode=DR,
                                     skip_group_check=True)
                for h in range(2):
                    nc.tensor.matmul(pc[:, h * HCH:(h + 1) * HCH], gw2c,
                                     z2c[:, :, h * HCH:(h + 1) * HCH],
                                     start=False, stop=True, perf_mode=DR,
                                     skip_group_check=True)

                # ---- squares (zq1 on ACT, zq2 mostly DVE; the engines run
                # the pair in parallel each chunk) ----
                zq1 = sqpool.tile([128, 2, CH], F8, tag="q1")
                zq2 = sqpool.tile([128, 2, CH], F8, tag="q2")
                nc.scalar.activation(out=zq1[:], in_=z1c, func=AF.Square)
                if c % 2 == 1:
                    nc.scalar.activation(out=zq2[:], in_=z2c,
                                         func=AF.Square)
                else:
                    nc.vector.scalar_tensor_tensor(
                        out=zq2[:], in0=z2c, scalar=1.0, in1=z2c,
                        op0=ALU.mult, op1=ALU.mult)

                # ---- mask + PSUM evac fused (DVE, spans both banks) ----
                mkc = mkpool.tile([128, CH], BF16, tag="mk")
                nc.vector.scalar_tensor_tensor(
                    out=mkc[:], in0=pc[:], scalar=1.0, in1=ohc,
                    op0=ALU.mult, op1=ALU.mult)

                stage[c] = (rwsc, rwcc, rn1c, rn2c, mkc, zq1, zq2)

            cc = c - 1
            if cc >= 0:
                rwsc, rwcc, rn1c, rn2c, mkc, zq1, zq2 = stage.pop(cc)
                for h in range(2):
                    first = cc == 0 and h == 0
                    last = cc == NCH - 1 and h == 1
                    hs = slice(h * HCH, (h + 1) * HCH)
                    nc.tensor.matmul(acc_s[:], rwsc[:, h, :], mkc[:, hs],
                                     start=first, stop=last,
                                     skip_group_check=True)
                    nc.tensor.matmul(acc_c[:], rwcc[:, h, :], mkc[:, hs],
                                     start=first, stop=last,
                                     skip_group_check=True)
                    nc.tensor.matmul(acc_n[:], rn1c[:, h], zq1[:, :, hs],
                                     start=first, stop=False, perf_mode=DR,
                                     skip_group_check=True)
                    nc.tensor.matmul(acc_n[:], rn2c[:, h], zq2[:, :, hs],
                                     start=False, stop=last, perf_mode=DR,
                                     skip_group_check=True)

        # ---- epilogue ----
        inv = singles.tile([NR, HCH], F32)
        nc.scalar.activation(out=inv[:], in_=acc_n[0:NR, :], func=AF.Ln,
                             bias=eps_b[:])
        nc.scalar.activation(out=inv[:], in_=inv[:], func=AF.Exp, scale=-0.5)
        sp = singles.tile([NR, HCH], F32)
        nc.vector.scalar_tensor_tensor(out=sp[:], in0=acc_s[:], scalar=1.0,
                                       in1=inv[:], op0=ALU.mult, op1=ALU.mult)
        cp = singles.tile([NR, HCH], F32)
        nc.vector.scalar_tensor_tensor(out=cp[:], in0=acc_c[:], scalar=1.0,
                                       in1=inv[:], op0=ALU.mult, op1=ALU.mult)
        # softplus(-x) = ln(1 + exp(-x)); Exp and Ln share one table set
        nc.scalar.activation(out=sp[:], in_=sp[:], func=AF.Exp, scale=-1.0)
        nc.scalar.activation(out=sp[:], in_=sp[:], func=AF.Ln, bias=one_b[:])
        nc.scalar.activation(out=cp[:], in_=cp[:], func=AF.Exp, scale=-1.0)
        nc.scalar.activation(out=cp[:], in_=cp[:], func=AF.Ln, bias=one_b[:])
        dts = singles.tile([NR, HCH], F32)
        nc.vector.scalar_tensor_tensor(out=dts[:], in0=cp[:], scalar=1.0,
                                       in1=sp[:], op0=ALU.mult,
                                       op1=ALU.subtract)
        jnk = singles.tile([NR, HCH], F32)
        accq = singles.tile([NR, 2], F32)
        nc.vector.scalar_tensor_tensor(out=jnk[:], in0=dts[:], scalar=1.0,
                                       in1=dts[:], op0=ALU.mult, op1=ALU.mult,
                                       accum_out=accq[:, 0:1])
        nc.vector.memset(accq[:, 1:2], 0.0)
        nc.sync.dma_start(acc[:], accq[:])

    nc.compile()
    return nc


# ---------------------------------------------------------------------------
# host-side prep
# ---------------------------------------------------------------------------

def _prep_core(z1s, z2s, b1s, b2s, g1n, g2n):
    """Build one core's packed input. z*s: [RPC, D] f32; b*s sorted int;
    g*n row-normalized [G, D] f32."""
    import ml_dtypes
    f8 = ml_dtypes.float8_e4m3
    bf = ml_dtypes.bfloat16

    def to_f8(x):
        return np.clip(x, -240.0, 240.0).astype(f8)

    nreal = z1s.shape[0]
    pkt = np.zeros((NCH, 128, CHB), np.uint8)

    def put(c, off, arr_bytes):
        pkt[c, :, off:off + arr_bytes.shape[1]] = arr_bytes

    z1p = np.zeros((NODES, D), np.float32)
    z1p[:nreal] = z1s
    z2p = np.zeros((NODES, D), np.float32)
    z2p[:nreal] = z2s
    # [NCH, 128, 2, CH] transposed fp8 (d = k*128 + p, k outer)
    z1t = to_f8(z1p.reshape(NCH, CH, 2, 128).transpose(0, 3, 2, 1))
    z2t = to_f8(z2p.reshape(NCH, CH, 2, 128).transpose(0, 3, 2, 1))

    v01 = np.zeros(NCH, np.int64)
    v02 = np.zeros(NCH, np.int64)
    for c in range(NCH):
        lo = min(c * CH, nreal - 1)
        hi = min((c + 1) * CH, nreal)
        v01[c] = b1s[lo]
        v02[c] = b2s[lo]
        if hi > c * CH:
            assert b1s[hi - 1] - v01[c] < W, f"chunk {c}: pair1 span"
            assert b2s[hi - 1] - v02[c] < W, f"chunk {c}: pair2 span"

    oh = np.zeros((NCH, 128, CH), np.float32)
    for c in range(NCH):
        hi = min((c + 1) * CH, nreal)
        nn = hi - c * CH
        if nn <= 0:
            continue
        idx = np.arange(nn)
        r1 = (b1s[c * CH:hi] - v01[c]).astype(np.int64)
        r2 = (b2s[c * CH:hi] - v02[c]).astype(np.int64)
        oh[c, r1, idx] = 1.0
        oh[c, W + r1, idx] = 1.0
        oh[c, 2 * W + r2, idx] = 1.0
        oh[c, 3 * W + r2, idx] = 1.0

    gw1 = np.zeros((NCH, 128, 2, 128), np.float32)
    gw2 = np.zeros((NCH, 128, 2, 128), np.float32)
    for c in range(NCH):
        for dst, blk, gn, v0 in ((gw1, 0, g1n, v01[c]), (gw1, 1, g2n, v01[c]),
                                 (gw2, 2, g2n, v02[c]), (gw2, 3, g1n, v02[c])):
            rows = np.minimum(v0 + np.arange(W), G - 1)
            gsel = gn[rows].T.reshape(2, 128, W).transpose(1, 0, 2)
            dst[c, :, :, blk * W:(blk + 1) * W] = gsel

    rws_np = np.zeros((NCH, 128, 2, NR), np.float32)
    rwc_np = np.zeros((NCH, 128, 2, NR), np.float32)
    rn1_np = np.zeros((NCH, 128, 2, 2, 32), np.float32)
    rn2_np = np.zeros((NCH, 128, 2, 2, 32), np.float32)
    for c in range(NCH):
        for h in range(2):
            r = 2 * c + h
            rws_np[c, 0:W, h, r] = 1.0
            rws_np[c, 2 * W:3 * W, h, 14 + r] = 1.0
            rwc_np[c, W:2 * W, h, r] = 1.0
            rwc_np[c, 3 * W:4 * W, h, 14 + r] = 1.0
            rn1_np[c, :, h, :, r] = 1.0
            rn2_np[c, :, h, :, 14 + r] = 1.0

    for c in range(NCH):
        put(c, OFF_Z1, z1t[c].reshape(128, 2 * CH).view(np.uint8))
        put(c, OFF_Z2, z2t[c].reshape(128, 2 * CH).view(np.uint8))
        put(c, OFF_OH, oh[c].astype(f8).view(np.uint8))
        put(c, OFF_GW1, to_f8(gw1[c]).reshape(128, 256).view(np.uint8))
        put(c, OFF_GW2, to_f8(gw2[c]).reshape(128, 256).view(np.uint8))
        put(c, OFF_RWS, rws_np[c].astype(bf).reshape(128, 56).view(np.uint8))
        put(c, OFF_RWC, rwc_np[c].astype(bf).reshape(128, 56).view(np.uint8))
        put(c, OFF_RN1, rn1_np[c].astype(f8).reshape(128, 128).view(np.uint8))
        put(c, OFF_RN2, rn2_np[c].astype(f8).reshape(128, 128).view(np.uint8))

    pkt = pkt.transpose(1, 0, 2).reshape(128, NCH * CHB)
    return {"pkt": np.ascontiguousarray(pkt).view(f8)}


def _prep_inputs(z1, z2, g1, g2, batch_1, batch_2):
    z1 = np.asarray(z1, np.float32)
    z2 = np.asarray(z2, np.float32)
    b1 = np.asarray(batch_1).astype(np.int64).ravel()
    b2 = np.asarray(batch_2).astype(np.int64).ravel()
    g1 = np.asarray(g1, np.float32)
    g2 = np.asarray(g2, np.float32)
    g1n = g1 / np.maximum(np.linalg.norm(g1, axis=1, keepdims=True), 1e-12)
    g2n = g2 / np.maximum(np.linalg.norm(g2, axis=1, keepdims=True), 1e-12)

    in_maps = []
    for k in range(NCORES):
        sl = slice(k * RPC, (k + 1) * RPC)
        in_maps.append(_prep_core(z1[sl], z2[sl], b1[sl], b2[sl], g1n, g2n))
    return in_maps


def _finish(results):
    t1 = 0.0
    t2 = 0.0
    for r in results:
        a = r["acc"].astype(np.float64).reshape(NR, 2).sum(1)
        t1 += a[0:14].sum()
        t2 += a[14:28].sum()
    return np.float32(np.sqrt(t1) + np.sqrt(t2))


_prog = None


def _get_prog():
    global _prog
    if _prog is None:
        _prog = build()
    return _prog


def kernel(z1, z2, g1, g2, batch_1, batch_2, trace=False):
    nc = _get_prog()
    in_maps = _prep_inputs(z1, z2, g1, g2, batch_1, batch_2)
    res = run_bass_kernel_spmd(nc, in_maps, core_ids=list(range(NCORES)),
                               trace=trace)
    out = _finish(res.results)
    if trace:
        kernel.last_results = res
    return out
